# revision 1
# baseline (speedup 1.0000x reference)
"""Trainium2 Bass kernel for a pre-LN transformer encoder block.

Model: y = x + FFN(LN2(x + Attn(LN1(x))))  with
  D_MODEL=1024, D_FF=4096, H=16 heads, B=4, S=2048, fp32.

Sharding (8 cores, zero collectives): core c handles batch b=c//2 and
query-half r=c%2.  Each core computes LN1 + K/V over its batch element's
full 2048 tokens (duplicated across the pair - cheaper than an
all-gather), all 16 heads of attention for its own 1024 queries, then
wo / LN2 / FFN for those 1024 tokens.  The token axis is rolled per core
so queries are always tokens 0..1023 -> one SPMD program for all cores.

On-device layout is transposed ([feature, token]) so projections feed
matmuls directly (contraction on partitions), biases are per-partition,
softmax denominators come from an appended ones-column on V, and the
attention mask folds into an extra contraction row of K.  All matmuls run
in float32r (TF32-like, full PE rate at free-dim >= 256).
"""

import numpy as np

D = 1024          # d_model
H = 16            # heads
DKH = 64          # head dim
DFF = 4096
T = 2048          # tokens per batch element (keys)
TQ = 1024         # queries per core
NEG = -1e9
EPS = 1e-5
P = 128

_CACHE = {}


def _build_nc():
    import concourse.bass as bass
    import concourse.tile as tile
    import concourse.mybir as mybir
    from concourse import bacc
    from concourse.bass import ts

    fp32 = mybir.dt.float32
    f32r = mybir.dt.float32r
    AF = mybir.ActivationFunctionType
    OP = mybir.AluOpType

    nc = bacc.Bacc("TRN2", target_bir_lowering=False, debug=False, num_devices=8)

    # ---- kernel I/O ----
    xT = nc.dram_tensor("xT", [D, T], f32r, kind="ExternalInput").ap()
    mrow = nc.dram_tensor("mrow", [2, T], f32r, kind="ExternalInput").ap()
    wq = nc.dram_tensor("wq", [D, D], f32r, kind="ExternalInput").ap()
    wk = nc.dram_tensor("wk", [D, D], f32r, kind="ExternalInput").ap()
    wv = nc.dram_tensor("wv", [D, D], f32r, kind="ExternalInput").ap()
    wo = nc.dram_tensor("wo", [D, D], f32r, kind="ExternalInput").ap()
    w1 = nc.dram_tensor("w1", [D, DFF], f32r, kind="ExternalInput").ap()
    w2 = nc.dram_tensor("w2", [DFF, D], f32r, kind="ExternalInput").ap()
    bqc = nc.dram_tensor("bqc", [P, 8], fp32, kind="ExternalInput").ap()
    bkc = nc.dram_tensor("bkc", [P, 8], fp32, kind="ExternalInput").ap()
    bvr = nc.dram_tensor("bvr", [1, D], f32r, kind="ExternalInput").ap()
    boc = nc.dram_tensor("boc", [P, 8], fp32, kind="ExternalInput").ap()
    b1c = nc.dram_tensor("b1c", [P, 32], fp32, kind="ExternalInput").ap()
    b2c = nc.dram_tensor("b2c", [P, 8], fp32, kind="ExternalInput").ap()
    ln1ab = nc.dram_tensor("ln1ab", [1, 2], fp32, kind="ExternalInput").ap()
    ln2ab = nc.dram_tensor("ln2ab", [1, 2], fp32, kind="ExternalInput").ap()
    yT = nc.dram_tensor("yT", [D, TQ], fp32, kind="ExternalOutput").ap()

    xTr = xT.rearrange("(c p) t -> p c t", p=P)       # [128, 8, 2048]
    w1r = w1.rearrange("(c p) f -> p c f", p=P)       # [128, 8, 4096]
    w2r = w2.rearrange("(j p) o -> p j o", p=P)       # [128, 32, 1024]
    yTr = yT.rearrange("(c p) t -> p c t", p=P)       # [128, 8, 1024]

    with tile.TileContext(nc) as tc:
        _emit(nc, tc, tile, mybir, ts, fp32, f32r, AF, OP, locals())
    nc.compile()
    return nc


def _emit(nc, tc, tile, mybir, ts, fp32, f32r, AF, OP, io):
    xT, xTr, mrow = io["xT"], io["xTr"], io["mrow"]
    wq, wk, wv, wo = io["wq"], io["wk"], io["wv"], io["wo"]
    w1r, w2r, yTr = io["w1r"], io["w2r"], io["yTr"]
    bqc, bkc, bvr, boc = io["bqc"], io["bkc"], io["bvr"], io["boc"]
    b1c, b2c, ln1ab, ln2ab = io["b1c"], io["b2c"], io["ln1ab"], io["ln2ab"]

    from contextlib import ExitStack
    es = ExitStack()
    with es:
        es.enter_context(nc.allow_low_precision(
            reason="float32r operands are deliberate; fp32 psum accumulation"))
        dram = es.enter_context(tc.tile_pool(name="dram", bufs=1, space="DRAM"))
        consts = es.enter_context(tc.tile_pool(name="consts", bufs=1))
        stg = es.enter_context(tc.tile_pool(name="stg", bufs=6))
        rows = es.enter_context(tc.tile_pool(name="rows", bufs=8))

        # DRAM scratch
        ktd = dram.tile([H, DKH + 1, T], f32r, tag="ktd")      # K^T + mask row
        qtd = dram.tile([H, DKH + 1, TQ], f32r, tag="qtd")     # Q^T + ones row
        ctxd = dram.tile([P, 8, TQ], f32r, tag="ctxd")         # ctx^T pair-chunked

        # ---- constants ----
        bq_sb = consts.tile([P, 8], fp32, tag="bq")
        nc.sync.dma_start(bq_sb[:], bqc[:])
        bk_sb = consts.tile([P, 8], fp32, tag="bk")
        nc.sync.dma_start(bk_sb[:], bkc[:])
        bo_sb = consts.tile([P, 8], fp32, tag="bo")
        nc.sync.dma_start(bo_sb[:], boc[:])
        b2_sb = consts.tile([P, 8], fp32, tag="b2")
        nc.sync.dma_start(b2_sb[:], b2c[:])
        b1_sb = consts.tile([P, 32], fp32, tag="b1")
        nc.sync.dma_start(b1_sb[:], b1c[:])
        bv_sb = consts.tile([P, D], f32r, tag="bv")            # bv broadcast on rows
        nc.sync.dma_start(bv_sb[:], bvr.to_broadcast((P, D)))
        ln1_sb = consts.tile([1, 2], fp32, tag="ln1")
        nc.sync.dma_start(ln1_sb[:], ln1ab[:])
        ln2_sb = consts.tile([1, 2], fp32, tag="ln2")
        nc.sync.dma_start(ln2_sb[:], ln2ab[:])
        # memset cannot write float32r directly; stage fp32 then DVE-copy
        ones_f = consts.tile([P, P], fp32, tag="ones_f")
        nc.vector.memset(ones_f[:], 1.0)
        ones_c = consts.tile([P, 1], f32r, tag="ones_c")       # colsum lhsT
        nc.vector.tensor_copy(ones_c[:], ones_f[:, 0:1])
        ones_r = consts.tile([1, P], f32r, tag="ones_r")       # bcast lhsT
        nc.vector.tensor_copy(ones_r[:], ones_f[0:1, :])

        # mask row of K^T and ones row of Q^T
        for h in range(H):
            nc.sync.dma_start(ktd[h, DKH : DKH + 1, :], mrow[0:1, :])
            nc.sync.dma_start(qtd[h, DKH : DKH + 1, :], mrow[1:2, 0:TQ])

        NT = T // 512   # 4 t-chunks of 512

        def layer_norm_cols(x_src_fn, ab_sb, sB_ps, tB_ps, psp):
            """Emit LN stats for one 512-token chunk.

            x_src_fn(c) -> [128, 512] f32r AP of input chunk c (c in 0..8).
            Fills sB_ps/tB_ps ([128,512] psum) with broadcast scale/shift:
            xn = x * sB - tB.
            """
            cx = psp.tile([1, 512], fp32, tag="sums", bufs=2)
            csq = psp.tile([1, 512], fp32, tag="sums", bufs=2)
            for c in range(8):
                nc.tensor.matmul(cx[:], ones_c[:], x_src_fn(c),
                                 start=(c == 0), stop=(c == 7))
            for c in range(8):
                sq = stg.tile([P, 512], f32r, tag="stg", name="sq")
                nc.vector.tensor_mul(sq[:], x_src_fn(c), x_src_fn(c))
                nc.tensor.matmul(csq[:], ones_c[:], sq[:],
                                 start=(c == 0), stop=(c == 7))
            mean = rows.tile([1, 512], fp32, tag="rows", name="mean")
            nc.vector.tensor_scalar_mul(mean[:], cx[:], 1.0 / D)
            m2s = rows.tile([1, 512], fp32, tag="rows", name="m2s")
            nc.vector.scalar_tensor_tensor(m2s[:], mean[:], float(D) / (D - 1),
                                           mean[:], op0=OP.mult, op1=OP.mult)
            var = rows.tile([1, 512], fp32, tag="rows", name="var")
            nc.vector.scalar_tensor_tensor(var[:], csq[:], 1.0 / (D - 1),
                                           m2s[:], op0=OP.mult, op1=OP.subtract)
            std = rows.tile([1, 512], fp32, tag="rows", name="std")
            nc.scalar.activation(std[:], var[:], AF.Sqrt)
            nc.vector.tensor_scalar_add(std[:], std[:], EPS)
            rstd = rows.tile([1, 512], fp32, tag="rows", name="rstd")
            nc.vector.reciprocal(rstd[:], std[:])
            s_r = rows.tile([1, 512], f32r, tag="rows", name="s_r")
            nc.vector.tensor_scalar_mul(s_r[:], rstd[:], ab_sb[0:1, 0:1])
            t_r = rows.tile([1, 512], f32r, tag="rows", name="t_r")
            nc.vector.tensor_mul(t_r[:], mean[:], s_r[:])
            nc.vector.tensor_scalar_sub(t_r[:], t_r[:], ab_sb[0:1, 1:2])
            nc.tensor.matmul(sB_ps[:], ones_r[:], s_r[:], start=True, stop=True)
            nc.tensor.matmul(tB_ps[:], ones_r[:], t_r[:], start=True, stop=True)

        # ================= P0: LN1 + Q/K/V projections =================
        with tc.tile_pool(name="p0big", bufs=1) as p0big:
            v_sb = p0big.tile([P, 16, H * (DKH + 1)], f32r, tag="vaug")
            # ones columns of V_aug (col 64 of each head block)
            vv = v_sb.rearrange("p t (h e) -> p t h e", e=DKH + 1)
            nc.vector.tensor_copy(
                vv[:, :, :, DKH : DKH + 1],
                ones_f[:, 0:1].to_broadcast((P, 16, H, 1)))

            with tc.tile_pool(name="p0", bufs=2) as p0, \
                 tc.tile_pool(name="ps0", bufs=1, space="PSUM") as ps0:
                for tci in range(NT):
                    tsl = ts(tci, 512)
                    x_sb = p0.tile([P, 8, 512], f32r, tag="xchunk")
                    nc.sync.dma_start(x_sb[:], xTr[:, :, tsl])
                    sB = ps0.tile([P, 512], fp32, tag="bcast", bufs=2)
                    tB = ps0.tile([P, 512], fp32, tag="bcast", bufs=2)
                    layer_norm_cols(lambda c: x_sb[:, c, :], ln1_sb, sB, tB, ps0)
                    xn_sb = p0.tile([P, 8, 512], f32r, tag="xnchunk")
                    for c in range(8):
                        nc.vector.tensor_mul(xn_sb[:, c, :], x_sb[:, c, :], sB[:])
                        nc.vector.tensor_sub(xn_sb[:, c, :], xn_sb[:, c, :], tB[:])

                    # K projection (transposed out) + optionally Q
                    for w_ap, b_sb, dst, ncols in (
                        (wk, bk_sb, ktd, T),
                        (wq, bq_sb, qtd, TQ),
                    ):
                        if tci * 512 >= ncols:
                            continue
                        wr = w_ap.rearrange("(c p) f -> p c f", p=P)
                        for dkb in range(2):
                            wb = p0.tile([P, 8, 512], f32r, tag="wblk")
                            nc.sync.dma_start(wb[:], wr[:, :, ts(dkb, 512)])
                            for dkc in range(4):
                                g = dkb * 4 + dkc
                                kps = ps0.tile([P, 512], fp32, tag="mm", bufs=4)
                                for c in range(8):
                                    nc.tensor.matmul(kps[:], wb[:, c, ts(dkc, P)],
                                                     xn_sb[:, c, :],
                                                     start=(c == 0), stop=(c == 7))
                                kst = stg.tile([P, 512], f32r, tag="stg", name="kst")
                                nc.vector.tensor_scalar_add(kst[:], kps[:],
                                                            b_sb[:, g : g + 1])
                                nc.sync.dma_start(dst[2 * g, 0:DKH, tsl],
                                                  kst[0:DKH, :])
                                nc.sync.dma_start(dst[2 * g + 1, 0:DKH, tsl],
                                                  kst[DKH:P, :])

                    # V projection (natural out), augmented layout
                    wvr = wv.rearrange("(c p) f -> p c f", p=P)
                    for dvb in range(2):
                        wb = p0.tile([P, 8, 512], f32r, tag="wblk")
                        nc.sync.dma_start(wb[:], wvr[:, :, ts(dvb, 512)])
                        for tsub in range(4):
                            tcc = tci * 4 + tsub
                            vps = ps0.tile([P, 512], fp32, tag="mm", bufs=4)
                            for c in range(8):
                                nc.tensor.matmul(vps[:], xn_sb[:, c, ts(tsub, P)],
                                                 wb[:, c, :],
                                                 start=(c == 0), stop=(c == 7))
                            vdst = v_sb[:, tcc, dvb * 8 * (DKH + 1) :
                                        (dvb + 1) * 8 * (DKH + 1)]
                            vdst = vdst.rearrange("p (h e) -> p h e", e=DKH + 1)
                            bsl = bv_sb[:, ts(dvb, 512)].rearrange(
                                "p (h e) -> p h e", e=DKH)
                            nc.vector.tensor_add(
                                vdst[:, :, 0:DKH],
                                vps.rearrange("p (h e) -> p h e", e=DKH),
                                bsl)

            # ================= P1: attention =================
            with tc.tile_pool(name="p1", bufs=2) as p1, \
                 tc.tile_pool(name="pr", bufs=4) as prp, \
                 tc.tile_pool(name="ps1", bufs=1, space="PSUM") as ps1:
                for h in range(H):
                    kt_sb = p1.tile([DKH + 1, T], f32r, tag="kt")
                    nc.sync.dma_start(kt_sb[:], ktd[h])
                    qh_sb = p1.tile([DKH + 1, TQ], f32r, tag="qh")
                    nc.sync.dma_start(qh_sb[:], qtd[h])
                    for qt in range(2):
                        qsl = ts(qt, 512)
                        ctx = ps1.tile([DKH + 1, 512], fp32, tag="ctx", bufs=2)
                        for kc2 in range(8):
                            sc = ps1.tile([P, 2, 512], fp32, tag="sc", bufs=2)
                            for j in range(2):
                                kc = 2 * kc2 + j
                                nc.tensor.matmul(sc[:, j, :],
                                                 kt_sb[:, ts(kc, P)],
                                                 qh_sb[:, qsl],
                                                 start=True, stop=True)
                            pr = prp.tile([P, 2, 512], f32r, tag="pr")
                            nc.scalar.activation(pr[:], sc[:], AF.Exp,
                                                 scale=1.0 / 8.0)
                            for j in range(2):
                                kc = 2 * kc2 + j
                                nc.tensor.matmul(
                                    ctx[:],
                                    v_sb[:, kc, h * (DKH + 1) : (h + 1) * (DKH + 1)],
                                    pr[:, j, :],
                                    start=(kc == 0), stop=(kc == 15))
                        # normalize by the denominator row and store ctx^T
                        rr = rows.tile([1, 512], f32r, tag="rows", name="rr")
                        nc.vector.reciprocal(rr[:], ctx[DKH : DKH + 1, :])
                        rb = ps1.tile([DKH, 512], fp32, tag="rb", bufs=2)
                        nc.tensor.matmul(rb[:], ones_r[0:1, 0:DKH], rr[:],
                                         start=True, stop=True)
                        cst = stg.tile([P, 512], f32r, tag="stg", name="cst")
                        nc.vector.tensor_copy(cst[0:DKH, :], ctx[0:DKH, :])
                        nc.vector.tensor_mul(cst[0:DKH, :], cst[0:DKH, :], rb[:])
                        nc.sync.dma_start(
                            ctxd[DKH * (h % 2) : DKH * (h % 2) + DKH, h // 2, qsl],
                            cst[0:DKH, :])

        # ================= P2: wo projection + residual =================
        with tc.tile_pool(name="p23", bufs=1) as p23:
            outT = p23.tile([P, 8, TQ], f32r, tag="outT")
            with tc.tile_pool(name="p2", bufs=1) as p2, \
                 tc.tile_pool(name="p2s", bufs=2) as p2s, \
                 tc.tile_pool(name="ps2", bufs=1, space="PSUM") as ps2:
                wo_sb = p2.tile([P, 8, D], f32r, tag="wo")
                nc.sync.dma_start(wo_sb[:], wo.rearrange("(c p) f -> p c f", p=P))
                for qt in range(2):
                    qsl = ts(qt, 512)
                    ccs = []
                    for c in range(8):
                        cc = p2s.tile([P, 512], f32r, tag="ctxc", bufs=10,
                                      name="cc")
                        nc.sync.dma_start(cc[:], ctxd[:, c, qsl])
                        ccs.append(cc)
                    for do in range(8):
                        ops_ = ps2.tile([P, 512], fp32, tag="mm", bufs=4)
                        for c in range(8):
                            nc.tensor.matmul(ops_[:], wo_sb[:, c, ts(do, P)],
                                             ccs[c][:],
                                             start=(c == 0), stop=(c == 7))
                        xq = p2s.tile([P, 512], f32r, tag="xq", bufs=2, name="xq")
                        nc.sync.dma_start(xq[:], xTr[:, do, qsl])
                        nc.vector.scalar_tensor_tensor(
                            outT[:, do, qsl], ops_[:], bo_sb[:, do : do + 1],
                            xq[:], op0=OP.add, op1=OP.add)

            # ================= P3: LN2 =================
            with tc.tile_pool(name="p3", bufs=1) as p3:
                xn2 = p3.tile([P, 8, TQ], f32r, tag="xn2")
                with tc.tile_pool(name="ps3", bufs=1, space="PSUM") as ps3:
                    for tci in range(2):
                        tsl = ts(tci, 512)
                        sB = ps3.tile([P, 512], fp32, tag="bcast", bufs=2)
                        tB = ps3.tile([P, 512], fp32, tag="bcast", bufs=2)
                        layer_norm_cols(lambda c: outT[:, c, tsl], ln2_sb,
                                        sB, tB, ps3)
                        for c in range(8):
                            nc.vector.tensor_mul(xn2[:, c, tsl],
                                                 outT[:, c, tsl], sB[:])
                            nc.vector.tensor_sub(xn2[:, c, tsl],
                                                 xn2[:, c, tsl], tB[:])

                # ================= P4: FFN + residual =================
                with tc.tile_pool(name="p4", bufs=1) as p4, \
                     tc.tile_pool(name="p4w", bufs=3) as p4w, \
                     tc.tile_pool(name="ps4", bufs=1, space="PSUM") as ps4:
                    h1_sb = p4.tile([P, 16, TQ], f32r, tag="h1")
                    for half in range(2):
                        # h1 = relu(w1^T xn2 + b1) for this dff half
                        for fb in range(8):           # 256-wide dff blocks
                            fof = half * 2048 + fb * 256
                            w1b = p4w.tile([P, 8, 256], f32r, tag="wstr",
                                           name="w1b")
                            nc.sync.dma_start(w1b[:],
                                              w1r[:, :, fof : fof + 256])
                            for fc in range(2):
                                f = fb * 2 + fc      # 0..15 within half
                                for qt in range(2):
                                    qsl = ts(qt, 512)
                                    hps = ps4.tile([P, 512], fp32, tag="h1m",
                                                   bufs=4)
                                    for c in range(8):
                                        nc.tensor.matmul(
                                            hps[:], w1b[:, c, ts(fc, P)],
                                            xn2[:, c, qsl],
                                            start=(c == 0), stop=(c == 7))
                                    nc.vector.tensor_scalar(
                                        h1_sb[:, f, qsl], hps[:],
                                        b1_sb[:, half * 16 + f : half * 16 + f + 1],
                                        0.0, op0=OP.add, op1=OP.max)
                        # h2 partial = w2^T h1 (+ b2 + residual on half 0)
                        for do in range(8):
                            w2c = p4w.tile([P, 16, P], f32r, tag="wstr",
                                           name="w2c")
                            nc.sync.dma_start(
                                w2c[:],
                                w2r[:, half * 16 : half * 16 + 16, ts(do, P)])
                            for qt in range(2):
                                qsl = ts(qt, 512)
                                h2p = ps4.tile([P, 512], fp32, tag="h2m", bufs=4)
                                for j in range(16):
                                    nc.tensor.matmul(h2p[:], w2c[:, j, :],
                                                     h1_sb[:, j, qsl],
                                                     start=(j == 0),
                                                     stop=(j == 15))
                                if half == 0:
                                    nc.vector.scalar_tensor_tensor(
                                        outT[:, do, qsl], h2p[:],
                                        b2_sb[:, do : do + 1],
                                        outT[:, do, qsl],
                                        op0=OP.add, op1=OP.add)
                                else:
                                    yst = stg.tile([P, 512], fp32, tag="stg",
                                                   name="yst")
                                    nc.vector.tensor_add(yst[:], h2p[:],
                                                         outT[:, do, qsl])
                                    nc.sync.dma_start(yTr[:, do, qsl], yst[:])


def _get_nc():
    if "nc" not in _CACHE:
        _CACHE["nc"] = _build_nc()
    return _CACHE["nc"]


def _make_in_maps(x, src_mask, wq, bq, wk, bk, wv, bv, wo, bo,
                  w1, b1, w2, b2, ln1_a, ln1_b, ln2_a, ln2_b):
    f = np.float32

    def chunk_bias(b, nc_):
        return np.ascontiguousarray(np.asarray(b, f).reshape(nc_, P).T)

    common = {
        "wq": np.ascontiguousarray(np.asarray(wq, f)),
        "wk": np.ascontiguousarray(np.asarray(wk, f)),
        "wv": np.ascontiguousarray(np.asarray(wv, f)),
        "wo": np.ascontiguousarray(np.asarray(wo, f)),
        "w1": np.ascontiguousarray(np.asarray(w1, f)),
        "w2": np.ascontiguousarray(np.asarray(w2, f)),
        "bqc": chunk_bias(bq, 8),
        "bkc": chunk_bias(bk, 8),
        "bvr": np.asarray(bv, f).reshape(1, D),
        "boc": chunk_bias(bo, 8),
        "b1c": chunk_bias(b1, 32),
        "b2c": chunk_bias(b2, 8),
        "ln1ab": np.array([[ln1_a.reshape(-1)[0], ln1_b.reshape(-1)[0]]], f),
        "ln2ab": np.array([[ln2_a.reshape(-1)[0], ln2_b.reshape(-1)[0]]], f),
    }
    in_maps = []
    for c in range(8):
        b, r = c // 2, c % 2
        xb = np.asarray(x[b], f)                       # [T, D]
        madd = np.where(np.asarray(src_mask[b]).reshape(T) == 0,
                        f(8.0 * NEG), f(0.0)).astype(f)
        ones = np.ones(T, f)
        if r:
            xb = np.concatenate([xb[TQ:], xb[:TQ]], axis=0)
            madd = np.concatenate([madd[TQ:], madd[:TQ]])
        m = dict(common)
        m["xT"] = np.ascontiguousarray(xb.T)           # [D, T]
        m["mrow"] = np.stack([madd, ones])             # [2, T]
        in_maps.append(m)
    return in_maps


def kernel(**inputs):
    from concourse import bass_utils

    nc = _get_nc()
    in_maps = _make_in_maps(**inputs)
    res = bass_utils.run_bass_kernel_spmd(nc, in_maps, core_ids=list(range(8)))
    B, S = 4, T
    out = np.empty((B, S, D), np.float32)
    for c in range(8):
        b, r = c // 2, c % 2
        out[b, r * TQ : (r + 1) * TQ, :] = res.results[c]["yT"].T
    return out

